# revision 44
# baseline (speedup 1.0000x reference)
"""Single-head self-attention (B=4, S=2048, D=1024, fp32) on 8 trn2 NeuronCores.

Sharding: each core owns (batch b = core//2, sequence half h = core%2), as in
the fp16 baseline: Q/K are folded into one projection via G = Wq Wk^T
(scores = x G x^T), V is computed for own j rows only, and each core emits
partial softmax numerator `pre` and denominator `den` for its j half; the
host combines halves exactly: out = (pre0 + pre1)/(den0 + den1) + bv
(bv is pulled out of the device: sum_j a_j (v_j + bv) = sum_j a_j v_j +
den * bv, which reduces to "+ bv" after the division).

Speed comes from fp8 DoubleRow matmuls (2 fp8 rows per PE pass, K=256 per
instruction). e4m3 alone is too coarse (~2.5% rms), so every matmul operand
X is carried as a split pair X = X8 (e4m3) + Xl (e5m2 residual, natural
scale) and each product uses three cross terms accumulated in one PSUM
group:  A@B ~= A8@B8 + A8@Bl + Al@B8  (the dropped Al@Bl term is O(delta^2)).
The scores phase further drops the last of 4 K-chunks of its key-side
residual term (KCB2=3) — measured end-to-end error 1.25e-2 (max metric,
rms 9.0e-3) vs the 2e-2 gate, saving 32 big matmuls/core. x/G/Wv
splits are host-prepared; M and attn splits are extracted on device from
PSUM via ACT copy (hi, e4m3) and DVE subtract (lo, e5m2). exp is shifted by
-3*ln2 (folded into the constant bias) so attn stays below e4m3 max; the
pre/den ratio is shift-invariant.

Schedule: a single 8-bank PSUM ring serves every phase (den/bias slice a
[P,1] column out of a full-bank tile). M runs first: wave 1 = all 8 jb0
groups kc-outer so matmuls chase the interleaved (g8, x8, xl, gl) quarter
stream in arrival order, with the last K-layer group-major so extracts
stagger; the remaining 8 jb1 groups run group-major with bias groups
interleaved after their extracts. B (scores) and C (attn @ V) are
software-pipelined (B0 B1 V C0 B2 C1 B3 C2 C3) to hide the exp/split
extraction latency; B0/B1 read only own-half q columns so the other x half
may arrive as late as B2. den values accumulate in a [P,16] SBUF tile and
leave in one end-of-kernel DMA; C3's last 512 output columns go as two
256-wide groups so the closing copy+DMA chain after the final matmul is
half-width. A few throwaway warm matmuls (NWARM=4) keep the first real
matmuls' cost-model visits past the 3us p-state ramp.
"""

import numpy as np
import ml_dtypes

import concourse.bass as bass
import concourse.mybir as mybir
import concourse.tile as tile
import concourse.tile_scheduler as _tsch
import concourse.tile_sem_assignment as _tsem
from concourse.bass_utils import run_bass_kernel_spmd

# 4 HWDGE queue semaphores instead of 8: fewer live sems shortens the
# end-of-kernel drain wait chain; 4 queues still keep DMA issue pipelined.
_tsch.NUM_HWDGE_SEMS = 4
_tsem.NUM_HWDGE_SEMS = 4

E4 = mybir.dt.float8e4
E5 = mybir.dt.float8e5
F16 = mybir.dt.float16
F32 = mybir.dt.float32
AFT = mybir.ActivationFunctionType
DR = mybir.MatmulPerfMode.DoubleRow
E4NP = ml_dtypes.float8_e4m3
E5NP = ml_dtypes.float8_e5m2

B, S, D = 4, 2048, 1024
NCORES = 8
P = 128
DC = D // P            # 8 contraction chunks of 128
KC = DC // 2           # 4 DoubleRow K-chunks of 256
JROWS = S // 2         # 1024 own k/v rows per core
JC = JROWS // P        # 8 own j chunks
QB = S // 512          # 4 query col-blocks of 512
SCALE = 1.0 / np.sqrt(np.float32(D))  # 1/32
ESHIFT = 3.0 * np.log(2.0)  # keep exp(score) < e4m3 max (448)

_CACHED = {}


def _split_excess_waits(nc, max_waits=1):
    """walrus in this env rejects >1 sync-wait per instruction (Drain at Tile
    exit carries one per live semaphore); move extras onto same-engine NOPs."""
    for f in nc.m.functions:
        for bb in f.blocks:
            new_list, changed = [], False
            for ins in bb.instructions:
                si = getattr(ins, "sync_info", None)
                ow = list(si.on_wait) if si and si.on_wait else []
                if len(ow) > max_waits:
                    extra, keep = ow[:-max_waits], ow[-max_waits:]
                    for k, w in enumerate(extra):
                        new_list.append(
                            mybir.InstNoOp(
                                name=f"{ins.name}_ws{k}",
                                engine=ins.engine,
                                sync_info=mybir.SyncInfo(on_wait=[w], on_update=[]),
                                bass_nofuse=True,
                            )
                        )
                    si.on_wait = keep
                    changed = True
                new_list.append(ins)
            if changed:
                bb.instructions = new_list


def _build():
    nc = bass.Bass("TRN2", target_bir_lowering=False, debug=False, num_devices=NCORES)

    # All big inputs are host pre-imaged to the SBUF layout [P, DC, X]
    # (partition p holds d = c*128+p) so each needs only two large DMAs:
    # HWDGE descriptor-generation time (625ns per DMA instruction, serialized)
    # gates the input stream, not bytes.
    x8o_d = nc.dram_tensor("x8o", [P, DC, JROWS], E4, kind="ExternalInput").ap()
    xlo_d = nc.dram_tensor("xlo", [P, DC, JROWS], E5, kind="ExternalInput").ap()
    x8r_d = nc.dram_tensor("x8r", [P, DC, JROWS], E4, kind="ExternalInput").ap()
    xlr_d = nc.dram_tensor("xlr", [P, DC, JROWS], E5, kind="ExternalInput").ap()
    g8_d = nc.dram_tensor("g8", [P, DC, D], E4, kind="ExternalInput").ap()
    gl_d = nc.dram_tensor("gl", [P, DC, D], E5, kind="ExternalInput").ap()
    wv8_d = nc.dram_tensor("wv8", [P, DC, D], E4, kind="ExternalInput").ap()
    wvl_d = nc.dram_tensor("wvl", [P, DC, D], E5, kind="ExternalInput").ap()
    w8_d = nc.dram_tensor("w8", [P, DC, 1], E4, kind="ExternalInput").ap()
    c0s_d = nc.dram_tensor("c0s", [P, 1], F32, kind="ExternalInput").ap()
    pre_d = nc.dram_tensor("pre", [S, D], F16, kind="ExternalOutput").ap()
    # den packed [P, 16]: col qb*4+qc holds den for query rows
    # qb*512+qc*128+p — one end-of-kernel DMA instead of 16 tiny ones
    den_d = nc.dram_tensor("den", [P, QB * 4], F32, kind="ExternalOutput").ap()

    with tile.TileContext(nc) as tc:
        with (
            tc.tile_pool(name="persist", bufs=1) as persist,
            tc.tile_pool(name="outp", bufs=16) as outp,
            tc.tile_pool(name="attnp", bufs=2) as attnp,
            tc.tile_pool(name="a16p", bufs=4) as a16p,
            tc.tile_pool(name="ps", bufs=8, space="PSUM") as psbig,
        ):
            # ---- persistent SBUF ----
            x8_sb = persist.tile([P, DC, S], E4, tag="x8")
            xl_sb = persist.tile([P, DC, S], E5, tag="xl")
            M8_sb = persist.tile([P, DC, JROWS], E4, tag="M8")
            Ml_sb = persist.tile([P, DC, JROWS], E5, tag="Ml")
            v8_sb = persist.tile([P, JC, D], E4, tag="v8")
            vl_sb = persist.tile([P, JC, D], E5, tag="vl")
            g8_sb = persist.tile([P, DC, D], E4, tag="g8")
            gl_sb = persist.tile([P, DC, D], E5, tag="gl")
            wv8_sb = persist.tile([P, DC, D], E4, tag="wv8")
            wvl_sb = persist.tile([P, DC, D], E5, tag="wvl")
            bqk_sb = persist.tile([P, JC], F32, tag="bqk")
            w8_sb = persist.tile([P, DC, 1], E4, tag="w8")
            c0s_sb = persist.tile([P, 1], F32, tag="c0s")
            ones_sb = persist.tile([P, JC, 1], E4, tag="ones")
            den_sb = persist.tile([P, QB * 4], F32, tag="den")

            nc.vector.memset(ones_sb, 1.0)

            # A few throwaway warm matmuls execute inside the initial DMA
            # wait; they push the first real matmuls' cost-model visit times
            # past the 3us p-state ramp so M runs at full PE clock.
            import os as _os
            # how many of B's 4 K-chunks keep the q-side (term 1) and
            # key-side (term 2) residual corrections. 3/4 drops a quarter of
            # the q-side correction: measured end-to-end error 1.50e-02 vs
            # the 2e-2 gate (3.52e-03 at 4/4), for 32 fewer big matmuls.
            _KCB1 = int(_os.environ.get("KCB1", "4"))
            _KCB2 = int(_os.environ.get("KCB2", "3"))
            _KCC1 = int(_os.environ.get("KCC1", "4"))
            _KCC2 = int(_os.environ.get("KCC2", "4"))
            _KCC2H = int(_os.environ.get("KCC2H", "0"))
            nwarm = int(_os.environ.get("NWARM", "4"))
            if nwarm:
                warm_sb = persist.tile([P, 512], F16, tag="warm")
                nc.vector.memset(warm_sb, 0.0)
                pw = psbig.tile([P, 512], F32, tag="ps")
                for _ in range(nwarm):
                    nc.tensor.matmul(
                        pw, warm_sb[:, 0:P], warm_sb, start=True, stop=True
                    )

            # DMA order == consumption order: M's four operand tensors
            # (g8, x8 own, xl own, gl) stream interleaved at quarter
            # granularity so M's kc-layers start as chunks land; the Wv pair
            # arrives during B0/B1 (V runs after B1); the other x half is
            # first read by B2, much later.
            for h in range(4):
                hs = slice(h * 2, h * 2 + 2)
                nc.sync.dma_start(out=g8_sb[:, hs, :], in_=g8_d[:, hs, :])
                nc.sync.dma_start(out=x8_sb[:, hs, 0:JROWS], in_=x8o_d[:, hs, :])
                nc.sync.dma_start(out=xl_sb[:, hs, 0:JROWS], in_=xlo_d[:, hs, :])
                nc.sync.dma_start(out=gl_sb[:, hs, :], in_=gl_d[:, hs, :])
            nc.sync.dma_start(out=w8_sb, in_=w8_d[:, :, :])
            nc.sync.dma_start(out=c0s_sb, in_=c0s_d[:, :])
            H = DC // 2
            for h in range(2):
                hs = slice(h * H, (h + 1) * H)
                nc.sync.dma_start(out=wv8_sb[:, hs, :], in_=wv8_d[:, hs, :])
                nc.sync.dma_start(out=wvl_sb[:, hs, :], in_=wvl_d[:, hs, :])
            for h in range(2):
                hs = slice(h * H, (h + 1) * H)
                nc.sync.dma_start(out=x8_sb[:, hs, JROWS:S], in_=x8r_d[:, hs, :])
                nc.sync.dma_start(out=xl_sb[:, hs, JROWS:S], in_=xlr_d[:, hs, :])

            def mm_term(ps, lhs, rhs, first, last):
                nc.tensor.matmul(ps, lhs, rhs, start=first, stop=last, perf_mode=DR)

            def extract(ps, hi_ap, lo_ap, halves=1):
                # halves=2 splits the copy/sub into column halves so the
                # PSUM slot releases after the shorter chained half
                w = ps.shape[-1] // halves
                for hh in range(halves):
                    sl = slice(hh * w, (hh + 1) * w)
                    nc.scalar.activation(
                        out=hi_ap[..., sl], in_=ps[:, sl], func=AFT.Copy, scale=1.0
                    )
                    nc.vector.tensor_tensor(
                        out=lo_ap[..., sl], in0=ps[:, sl], in1=hi_ap[..., sl],
                        op=mybir.AluOpType.subtract,
                    )

            # bias[j] = (x_j.w)*SCALE + (bq.bk*SCALE - 3ln2), [j, 1]; its
            # tiny psum groups are interleaved between M group extractions so
            # their pool-recycle latency hides under matmul work.
            def bias_group(j):
                js = slice(j * P, (j + 1) * P)
                pbt = psbig.tile([P, 512], F32, tag="ps")
                pb = pbt[:, 0:1]
                for kc in range(KC):
                    nc.tensor.matmul(
                        pb,
                        x8_sb[:, 2 * kc : 2 * kc + 2, js],
                        w8_sb[:, 2 * kc : 2 * kc + 2, :],
                        start=(kc == 0),
                        stop=(kc == KC - 1),
                        perf_mode=DR,
                    )
                nc.vector.tensor_scalar(
                    out=bqk_sb[:, j : j + 1], in0=pb,
                    scalar1=float(SCALE), scalar2=c0s_sb[:, 0:1],
                    op0=mybir.AluOpType.mult, op1=mybir.AluOpType.add,
                )

            # ---- phase M: M[d, j] = sum_d' G[d,d'] x[j,d'] (own j) ----
            # Wave 1 = all 8 jb0 groups, kc-outer through layer KC-2 so each
            # K-chunk's matmuls run as the chunk quad (g8, x8, xl, gl) lands;
            # the last layer runs group-major so extracts stagger into the
            # following groups' matmuls instead of bunching at a wave barrier.
            # Remaining 8 groups (jb1) run group-major: extract right after
            # each group's 12 matmuls, pipelined with the next group.
            mgroups = [(jb, m) for jb in range(JROWS // 512) for m in range(DC)]
            wave1, rest = mgroups[:8], mgroups[8:]

            def m_ap(g):
                jb, m = g
                rs = slice(jb * 512, (jb + 1) * 512)
                ms = slice(m * P, (m + 1) * P)
                return rs, ms

            gidx = 0
            tiles = {
                g: psbig.tile([P, 512], F32, tag="ps", name=f"mps{g[1]}")
                for g in wave1
            }
            # kc-outer through layer KC-2 (consume the stream in arrival
            # order); the last layer runs group-major so each group stops
            # early and its extract staggers into the following matmuls.
            for kc in range(KC - 1):
                for term in range(3):
                    for g in wave1:
                        rs, ms = m_ap(g)
                        ks = slice(2 * kc, 2 * kc + 2)
                        lhs = (gl_sb if term == 2 else g8_sb)[:, ks, ms]
                        rhs = (xl_sb if term == 1 else x8_sb)[:, ks, rs]
                        mm_term(tiles[g], lhs, rhs, kc == 0 and term == 0, False)
            ks = slice(2 * (KC - 1), 2 * KC)
            for g in wave1:
                rs, ms = m_ap(g)
                for term in range(3):
                    lhs = (gl_sb if term == 2 else g8_sb)[:, ks, ms]
                    rhs = (xl_sb if term == 1 else x8_sb)[:, ks, rs]
                    mm_term(tiles[g], lhs, rhs, False, term == 2)
                jb, m = g
                with tc.high_priority():
                    extract(tiles[g], M8_sb[:, m, rs], Ml_sb[:, m, rs], halves=2)
                gidx += 1
            for g in rest:
                rs, ms = m_ap(g)
                wps = psbig.tile([P, 512], F32, tag="ps")
                n = 0
                for kc in range(KC):
                    for term in range(3):
                        ksg = slice(2 * kc, 2 * kc + 2)
                        lhs = (gl_sb if term == 2 else g8_sb)[:, ksg, ms]
                        rhs = (xl_sb if term == 1 else x8_sb)[:, ksg, rs]
                        mm_term(wps, lhs, rhs, n == 0, n == 3 * KC - 1)
                        n += 1
                jb, m = g
                extract(wps, M8_sb[:, m, rs], Ml_sb[:, m, rs])
                if gidx - 8 < JC:
                    bias_group(gidx - 8)
                gidx += 1

            # ---- phase V: v = x @ Wv (own j; bv folded out on host) ----
            # Runs between B1 and C0 (its first consumer): by then all of its
            # DMA has long landed, so plain group order, no stalls.
            def phase_v():
                for j in range(JC):
                    js = slice(j * P, (j + 1) * P)
                    for ob in range(2):
                        os_ = slice(ob * 512, (ob + 1) * 512)
                        ps = psbig.tile([P, 512], F32, tag="ps")
                        n = 0
                        for term in range(3):
                            for kc in range(KC):
                                ks = slice(2 * kc, 2 * kc + 2)
                                lhs = (xl_sb if term == 2 else x8_sb)[:, ks, js]
                                rhs = (wvl_sb if term == 1 else wv8_sb)[:, ks, os_]
                                mm_term(ps, lhs, rhs, n == 0, n == 3 * KC - 1)
                                n += 1
                        extract(ps, v8_sb[:, j, os_], vl_sb[:, j, os_])

            # ---- phases B+C, software-pipelined over query blocks ----
            ahis, alos = {}, {}

            def phase_b(qb):
                qs = slice(qb * 512, (qb + 1) * 512)
                ahi = attnp.tile([P, JC, 512], E4, tag="ahi")
                alo = attnp.tile([P, JC, 512], E5, tag="alo")
                ahis[qb], alos[qb] = ahi, alo
                for j in range(JC):
                    js = slice(j * P, (j + 1) * P)
                    ps = psbig.tile([P, 512], F32, tag="ps")
                    mms = [
                        (term, kc)
                        for term in range(3)
                        for kc in range(KC)
                        if not (term == 1 and kc >= _KCB1)
                        and not (term == 2 and kc >= _KCB2)
                    ]
                    for n, (term, kc) in enumerate(mms):
                        ks = slice(2 * kc, 2 * kc + 2)
                        lhs = (Ml_sb if term == 2 else M8_sb)[:, ks, js]
                        rhs = (xl_sb if term == 1 else x8_sb)[:, ks, qs]
                        mm_term(ps, lhs, rhs, n == 0, n == len(mms) - 1)
                    a16 = a16p.tile([P, 512], F16, tag="a16")
                    nc.scalar.activation(
                        out=a16, in_=ps, func=AFT.Exp,
                        scale=float(SCALE), bias=bqk_sb[:, j : j + 1],
                    )
                    nc.scalar.activation(
                        out=ahi[:, j, :], in_=a16, func=AFT.Copy, scale=1.0
                    )
                    nc.vector.tensor_tensor(
                        out=alo[:, j, :], in0=a16, in1=ahi[:, j, :],
                        op=mybir.AluOpType.subtract,
                    )

            def phase_c(qb):
                ahi, alo = ahis.pop(qb), alos.pop(qb)
                for qc in range(4):
                    qls = slice(qc * P, (qc + 1) * P)
                    pdt = psbig.tile([P, 512], F32, tag="ps")
                    pd = pdt[:, 0:1]
                    for kc in range(KC):
                        nc.tensor.matmul(
                            pd, ahi[:, 2 * kc : 2 * kc + 2, qls],
                            ones_sb[:, 2 * kc : 2 * kc + 2, :],
                            start=(kc == 0), stop=False, perf_mode=DR,
                        )
                    for kc in range(KC):
                        nc.tensor.matmul(
                            pd, alo[:, 2 * kc : 2 * kc + 2, qls],
                            ones_sb[:, 2 * kc : 2 * kc + 2, :],
                            start=False, stop=(kc == KC - 1), perf_mode=DR,
                        )
                    qrow = qb * 512 + qc * P
                    di = qb * 4 + qc
                    nc.vector.tensor_copy(den_sb[:, di : di + 1], pd)
                    last = qb == 3 and qc == 3
                    for ob in range(2):
                        if last and ob == 1:
                            continue
                        os_ = slice(ob * 512, (ob + 1) * 512)
                        po = psbig.tile([P, 512], F32, tag="ps")
                        cmms = [
                            (term, kc)
                            for term in range(3)
                            for kc in range(KC)
                            if not (term == 1 and kc >= _KCC1)
                            and not (
                                term == 2 and kc >= _KCC2
                                and (ob == 1 or not _KCC2H)
                            )
                        ]
                        for n, (term, kc) in enumerate(cmms):
                            ks = slice(2 * kc, 2 * kc + 2)
                            lhs = (alo if term == 2 else ahi)[:, ks, qls]
                            rhs = (vl_sb if term == 1 else v8_sb)[:, ks, os_]
                            mm_term(po, lhs, rhs, n == 0, n == len(cmms) - 1)
                        o = outp.tile([P, 512], F16, tag="o")
                        # alternate copies between DVE and ACT: halves the
                        # serial copy chain at the kernel tail and balances
                        # elementwise load across engines during C phases
                        # producer engine also issues the DMA: keeps output
                        # DMA issue off the SP queue (which would serialize
                        # the kernel tail behind 650ns/DMA SEQ holds)
                        if ob == 0:
                            nc.vector.tensor_copy(o, po)
                            nc.sync.dma_start(out=pre_d[qrow : qrow + P, os_], in_=o)
                        else:
                            nc.scalar.activation(out=o, in_=po, func=AFT.Copy, scale=1.0)
                            nc.scalar.dma_start(out=pre_d[qrow : qrow + P, os_], in_=o)
                    if last:
                        # final 512 cols go as two 256-wide groups so the
                        # closing copy+DMA chain after the very last matmul
                        # is half-width (and the last DMA rides SP's shorter
                        # DGE delay)
                        for half in range(2):
                            os2 = slice(512 + half * 256, 512 + (half + 1) * 256)
                            po2 = psbig.tile([P, 256], F32, tag="ps")
                            cmms = [
                                (term, kc)
                                for term in range(3)
                                for kc in range(KC)
                                if not (term == 1 and kc >= _KCC1)
                                and not (term == 2 and kc >= _KCC2)
                            ]
                            for n, (term, kc) in enumerate(cmms):
                                ks = slice(2 * kc, 2 * kc + 2)
                                lhs = (alo if term == 2 else ahi)[:, ks, qls]
                                rhs = (vl_sb if term == 1 else v8_sb)[:, ks, os2]
                                mm_term(po2, lhs, rhs, n == 0, n == len(cmms) - 1)
                            o2 = outp.tile([P, 256], F16, tag="o2", bufs=2)
                            if half == 0:
                                nc.scalar.activation(
                                    out=o2, in_=po2, func=AFT.Copy, scale=1.0
                                )
                                nc.scalar.dma_start(
                                    out=pre_d[qrow : qrow + P, os2], in_=o2
                                )
                            else:
                                nc.vector.tensor_copy(o2, po2)
                                nc.sync.dma_start(
                                    out=pre_d[qrow : qrow + P, os2], in_=o2
                                )

            phase_b(0)
            phase_b(1)
            phase_v()
            phase_c(0)
            phase_b(2)
            phase_c(1)
            phase_b(3)
            phase_c(2)
            phase_c(3)
            nc.sync.dma_start(out=den_d, in_=den_sb)

    _split_excess_waits(nc)
    return nc


def _get_nc():
    if "nc" not in _CACHED:
        _CACHED["nc"] = _build()
    return _CACHED["nc"]


def _split8(a):
    hi = np.ascontiguousarray(a).astype(E4NP)
    lo = (a - hi.astype(np.float32)).astype(E5NP)
    return hi, np.ascontiguousarray(lo)


def _img(a):
    """[D, X] -> SBUF image [P, DC, X] (partition p holds row d = c*128+p)."""
    return np.ascontiguousarray(a.reshape(DC, P, -1).swapaxes(0, 1))


def kernel(x, Wq, bq, Wk, bk, Wv, bv):
    x = np.asarray(x, dtype=np.float32)
    Wq32 = np.asarray(Wq, np.float32)
    Wk32 = np.asarray(Wk, np.float32)
    bq32 = np.asarray(bq, np.float32)
    bk32 = np.asarray(bk, np.float32)
    # weight fusion: G^T = Wk Wq^T so scores = x G x^T; w = Wk bq; c0 = bq.bk
    g8, gl = _split8(Wk32 @ Wq32.T)
    g8, gl = _img(g8), _img(gl)
    wv8, wvl = _split8(np.asarray(Wv, np.float32))
    wv8, wvl = _img(wv8), _img(wvl)
    w8 = np.ascontiguousarray(
        (Wk32 @ bq32).reshape(DC, P).T.reshape(P, DC, 1)
    ).astype(E4NP)
    c0s = np.full(
        (P, 1), float(SCALE) * float(bq32 @ bk32) - ESHIFT, np.float32
    )
    bv32 = np.asarray(bv, np.float32).reshape(1, 1, D)

    in_maps = []
    for core in range(NCORES):
        b, h = core // 2, core % 2
        # own j rows first (j order is internal; q order is undone on gather)
        xb = np.roll(x[b], -h * JROWS, axis=0) if h else x[b]
        x8, xlo = _split8(np.ascontiguousarray(xb.T))  # [D, S]
        x8i, xli = _img(x8), _img(xlo)  # [P, DC, S]
        in_maps.append(
            {"x8o": np.ascontiguousarray(x8i[:, :, 0:JROWS]),
             "xlo": np.ascontiguousarray(xli[:, :, 0:JROWS]),
             "x8r": np.ascontiguousarray(x8i[:, :, JROWS:S]),
             "xlr": np.ascontiguousarray(xli[:, :, JROWS:S]),
             "g8": g8, "gl": gl, "wv8": wv8, "wvl": wvl,
             "w8": w8, "c0s": c0s}
        )

    res = run_bass_kernel_spmd(_get_nc(), in_maps, list(range(NCORES)))
    out = np.empty((B, S, D), np.float32)
    for b in range(B):
        r0, r1 = res.results[2 * b], res.results[2 * b + 1]
        pre = r0["pre"].astype(np.float32) + np.roll(
            r1["pre"].astype(np.float32), JROWS, axis=0
        )
        # den comes back packed [P, 16]: col qb*4+qc = rows qb*512+qc*128+p
        d0 = np.ascontiguousarray(r0["den"].T).reshape(S, 1)
        d1 = np.ascontiguousarray(r1["den"].T).reshape(S, 1)
        den = d0 + np.roll(d1, JROWS, axis=0)
        out[b] = pre / den
    out += bv32
    return out



# revision 45
# speedup vs baseline: 1.0503x; 1.0503x over previous
"""Single-head self-attention (B=4, S=2048, D=1024, fp32) on 8 trn2 NeuronCores.

Sharding: each core owns (batch b = core//2, sequence half h = core%2), as in
the fp16 baseline: Q/K are folded into one projection via G = Wq Wk^T
(scores = x G x^T), V is computed for own j rows only, and each core emits
partial softmax numerator `pre` and denominator `den` for its j half; the
host combines halves exactly: out = (pre0 + pre1)/(den0 + den1) + bv
(bv is pulled out of the device: sum_j a_j (v_j + bv) = sum_j a_j v_j +
den * bv, which reduces to "+ bv" after the division).

Speed comes from fp8 DoubleRow matmuls (2 fp8 rows per PE pass, K=256 per
instruction). e4m3 alone is too coarse (~2.5% rms), so every matmul operand
X is carried as a split pair X = X8 (e4m3) + Xl (e5m2 residual, natural
scale) and each product uses three cross terms accumulated in one PSUM
group:  A@B ~= A8@B8 + A8@Bl + Al@B8  (the dropped Al@Bl term is O(delta^2)).
The scores phase further drops the last of 4 K-chunks of its key-side
residual term (KCB2=3) — measured end-to-end error 1.25e-2 (max metric,
rms 9.0e-3) vs the 2e-2 gate, saving 32 big matmuls/core. x/G/Wv
splits are host-prepared; M and attn splits are extracted on device from
PSUM via ACT copy (hi, e4m3) and DVE subtract (lo, e5m2). exp is shifted by
-3*ln2 (folded into the constant bias) so attn stays below e4m3 max; the
pre/den ratio is shift-invariant.

Schedule: a single 8-bank PSUM ring serves every phase (den/bias slice a
[P,1] column out of a full-bank tile). M runs first: wave 1 = all 8 jb0
groups kc-outer so matmuls chase the interleaved (g8, x8, xl, gl) quarter
stream in arrival order, with the last K-layer group-major so extracts
stagger; the remaining 8 jb1 groups run group-major with bias groups
interleaved after their extracts. B (scores) and C (attn @ V) are
software-pipelined (B0 B1 V C0 B2 C1 B3 C2 C3) to hide the exp/split
extraction latency; B0/B1 read only own-half q columns so the other x half
may arrive as late as B2. den values accumulate in a [P,16] SBUF tile and
leave in one end-of-kernel DMA; C3's last 512 output columns go as two
256-wide groups so the closing copy+DMA chain after the final matmul is
half-width. A few throwaway warm matmuls (NWARM=4) keep the first real
matmuls' cost-model visits past the 3us p-state ramp.
"""

import numpy as np
import ml_dtypes

import concourse.bass as bass
import concourse.mybir as mybir
import concourse.tile as tile
import concourse.tile_scheduler as _tsch
import concourse.tile_sem_assignment as _tsem
from concourse.bass_utils import run_bass_kernel_spmd

# 4 HWDGE queue semaphores instead of 8: fewer live sems shortens the
# end-of-kernel drain wait chain; 4 queues still keep DMA issue pipelined.
_tsch.NUM_HWDGE_SEMS = 4
_tsem.NUM_HWDGE_SEMS = 4

E4 = mybir.dt.float8e4
E5 = mybir.dt.float8e5
F16 = mybir.dt.float16
F32 = mybir.dt.float32
AFT = mybir.ActivationFunctionType
DR = mybir.MatmulPerfMode.DoubleRow
E4NP = ml_dtypes.float8_e4m3
E5NP = ml_dtypes.float8_e5m2

B, S, D = 4, 2048, 1024
NCORES = 8
P = 128
DC = D // P            # 8 contraction chunks of 128
KC = DC // 2           # 4 DoubleRow K-chunks of 256
JROWS = S // 2         # 1024 own k/v rows per core
JC = JROWS // P        # 8 own j chunks
QB = S // 512          # 4 query col-blocks of 512
SCALE = 1.0 / np.sqrt(np.float32(D))  # 1/32
ESHIFT = 3.0 * np.log(2.0)  # keep exp(score) < e4m3 max (448)

_CACHED = {}


def _split_excess_waits(nc, max_waits=1):
    """walrus in this env rejects >1 sync-wait per instruction (Drain at Tile
    exit carries one per live semaphore); move extras onto same-engine NOPs."""
    for f in nc.m.functions:
        for bb in f.blocks:
            new_list, changed = [], False
            for ins in bb.instructions:
                si = getattr(ins, "sync_info", None)
                ow = list(si.on_wait) if si and si.on_wait else []
                if len(ow) > max_waits:
                    extra, keep = ow[:-max_waits], ow[-max_waits:]
                    for k, w in enumerate(extra):
                        new_list.append(
                            mybir.InstNoOp(
                                name=f"{ins.name}_ws{k}",
                                engine=ins.engine,
                                sync_info=mybir.SyncInfo(on_wait=[w], on_update=[]),
                                bass_nofuse=True,
                            )
                        )
                    si.on_wait = keep
                    changed = True
                new_list.append(ins)
            if changed:
                bb.instructions = new_list


def _build():
    nc = bass.Bass("TRN2", target_bir_lowering=False, debug=False, num_devices=NCORES)

    # All big inputs are host pre-imaged to the SBUF layout [P, DC, X]
    # (partition p holds d = c*128+p) so each needs only two large DMAs:
    # HWDGE descriptor-generation time (625ns per DMA instruction, serialized)
    # gates the input stream, not bytes.
    x8o_d = nc.dram_tensor("x8o", [P, DC, JROWS], E4, kind="ExternalInput").ap()
    xlo_d = nc.dram_tensor("xlo", [P, DC, JROWS], E5, kind="ExternalInput").ap()
    x8r_d = nc.dram_tensor("x8r", [P, DC, JROWS], E4, kind="ExternalInput").ap()
    xlr_d = nc.dram_tensor("xlr", [P, DC, JROWS], E5, kind="ExternalInput").ap()
    g8_d = nc.dram_tensor("g8", [P, DC, D], E4, kind="ExternalInput").ap()
    gl_d = nc.dram_tensor("gl", [P, DC, D], E5, kind="ExternalInput").ap()
    wv8_d = nc.dram_tensor("wv8", [P, DC, D], E4, kind="ExternalInput").ap()
    wvl_d = nc.dram_tensor("wvl", [P, DC, D], E5, kind="ExternalInput").ap()
    w8_d = nc.dram_tensor("w8", [P, DC, 1], E4, kind="ExternalInput").ap()
    c0s_d = nc.dram_tensor("c0s", [P, 1], F32, kind="ExternalInput").ap()
    pre_d = nc.dram_tensor("pre", [S, D], F16, kind="ExternalOutput").ap()
    # den packed [P, 16]: col qb*4+qc holds den for query rows
    # qb*512+qc*128+p — one end-of-kernel DMA instead of 16 tiny ones
    den_d = nc.dram_tensor("den", [P, QB * 4], F32, kind="ExternalOutput").ap()

    with tile.TileContext(nc) as tc:
        with (
            tc.tile_pool(name="persist", bufs=1) as persist,
            tc.tile_pool(name="outp", bufs=16) as outp,
            tc.tile_pool(name="attnp", bufs=2) as attnp,
            tc.tile_pool(name="a16p", bufs=4) as a16p,
            tc.tile_pool(name="ps", bufs=8, space="PSUM") as psbig,
        ):
            # ---- persistent SBUF ----
            x8_sb = persist.tile([P, DC, S], E4, tag="x8")
            xl_sb = persist.tile([P, DC, S], E5, tag="xl")
            M8_sb = persist.tile([P, DC, JROWS], E4, tag="M8")
            Ml_sb = persist.tile([P, DC, JROWS], E5, tag="Ml")
            v8_sb = persist.tile([P, JC, D], E4, tag="v8")
            vl_sb = persist.tile([P, JC, D], E5, tag="vl")
            g8_sb = persist.tile([P, DC, D], E4, tag="g8")
            gl_sb = persist.tile([P, DC, D], E5, tag="gl")
            wv8_sb = persist.tile([P, DC, D], E4, tag="wv8")
            wvl_sb = persist.tile([P, DC, D], E5, tag="wvl")
            bqk_sb = persist.tile([P, JC], F32, tag="bqk")
            w8_sb = persist.tile([P, DC, 1], E4, tag="w8")
            c0s_sb = persist.tile([P, 1], F32, tag="c0s")
            ones_sb = persist.tile([P, JC, 1], E4, tag="ones")
            den_sb = persist.tile([P, QB * 4], F32, tag="den")

            nc.vector.memset(ones_sb, 1.0)

            # A few throwaway warm matmuls execute inside the initial DMA
            # wait; they push the first real matmuls' cost-model visit times
            # past the 3us p-state ramp so M runs at full PE clock.
            import os as _os
            # how many of B's 4 K-chunks keep the q-side (term 1) and
            # key-side (term 2) residual corrections. 3/4 drops a quarter of
            # the q-side correction: measured end-to-end error 1.50e-02 vs
            # the 2e-2 gate (3.52e-03 at 4/4), for 32 fewer big matmuls.
            _KCB1 = int(_os.environ.get("KCB1", "4"))
            _KCB2 = int(_os.environ.get("KCB2", "3"))
            _KCC1 = int(_os.environ.get("KCC1", "4"))
            _KCC2 = int(_os.environ.get("KCC2", "4"))
            _KCC2H = int(_os.environ.get("KCC2H", "0"))
            nwarm = int(_os.environ.get("NWARM", "4"))
            if nwarm:
                warm_sb = persist.tile([P, 512], F16, tag="warm")
                nc.vector.memset(warm_sb, 0.0)
                pw = psbig.tile([P, 512], F32, tag="ps")
                for _ in range(nwarm):
                    nc.tensor.matmul(
                        pw, warm_sb[:, 0:P], warm_sb, start=True, stop=True
                    )

            # DMA order == consumption order: M's four operand tensors
            # (g8, x8 own, xl own, gl) stream interleaved at quarter
            # granularity so M's kc-layers start as chunks land; the Wv pair
            # arrives during B0/B1 (V runs after B1); the other x half is
            # first read by B2, much later.
            for h in range(4):
                hs = slice(h * 2, h * 2 + 2)
                nc.sync.dma_start(out=g8_sb[:, hs, :], in_=g8_d[:, hs, :])
                nc.sync.dma_start(out=x8_sb[:, hs, 0:JROWS], in_=x8o_d[:, hs, :])
                nc.sync.dma_start(out=xl_sb[:, hs, 0:JROWS], in_=xlo_d[:, hs, :])
                nc.sync.dma_start(out=gl_sb[:, hs, :], in_=gl_d[:, hs, :])
            nc.sync.dma_start(out=w8_sb, in_=w8_d[:, :, :])
            nc.sync.dma_start(out=c0s_sb, in_=c0s_d[:, :])
            H = DC // 2
            for h in range(2):
                hs = slice(h * H, (h + 1) * H)
                nc.sync.dma_start(out=wv8_sb[:, hs, :], in_=wv8_d[:, hs, :])
                nc.sync.dma_start(out=wvl_sb[:, hs, :], in_=wvl_d[:, hs, :])
            for h in range(2):
                hs = slice(h * H, (h + 1) * H)
                nc.sync.dma_start(out=x8_sb[:, hs, JROWS:S], in_=x8r_d[:, hs, :])
                nc.sync.dma_start(out=xl_sb[:, hs, JROWS:S], in_=xlr_d[:, hs, :])

            def mm_term(ps, lhs, rhs, first, last):
                nc.tensor.matmul(ps, lhs, rhs, start=first, stop=last, perf_mode=DR)

            def extract(ps, hi_ap, lo_ap, halves=1):
                # halves=2 splits the copy/sub into column halves so the
                # PSUM slot releases after the shorter chained half
                w = ps.shape[-1] // halves
                for hh in range(halves):
                    sl = slice(hh * w, (hh + 1) * w)
                    nc.scalar.activation(
                        out=hi_ap[..., sl], in_=ps[:, sl], func=AFT.Copy, scale=1.0
                    )
                    nc.vector.tensor_tensor(
                        out=lo_ap[..., sl], in0=ps[:, sl], in1=hi_ap[..., sl],
                        op=mybir.AluOpType.subtract,
                    )

            # bias[j] = (x_j.w)*SCALE + (bq.bk*SCALE - 3ln2), [j, 1]; its
            # tiny psum groups are interleaved between M group extractions so
            # their pool-recycle latency hides under matmul work.
            def bias_group(j):
                js = slice(j * P, (j + 1) * P)
                pbt = psbig.tile([P, 512], F32, tag="ps")
                pb = pbt[:, 0:1]
                for kc in range(KC):
                    nc.tensor.matmul(
                        pb,
                        x8_sb[:, 2 * kc : 2 * kc + 2, js],
                        w8_sb[:, 2 * kc : 2 * kc + 2, :],
                        start=(kc == 0),
                        stop=(kc == KC - 1),
                        perf_mode=DR,
                    )
                nc.vector.tensor_scalar(
                    out=bqk_sb[:, j : j + 1], in0=pb,
                    scalar1=float(SCALE), scalar2=c0s_sb[:, 0:1],
                    op0=mybir.AluOpType.mult, op1=mybir.AluOpType.add,
                )

            # ---- phase M: M[d, j] = sum_d' G[d,d'] x[j,d'] (own j) ----
            # Wave 1 = all 8 jb0 groups, kc-outer through layer KC-2 so each
            # K-chunk's matmuls run as the chunk quad (g8, x8, xl, gl) lands;
            # the last layer runs group-major so extracts stagger into the
            # following groups' matmuls instead of bunching at a wave barrier.
            # Remaining 8 groups (jb1) run group-major: extract right after
            # each group's 12 matmuls, pipelined with the next group.
            mgroups = [(jb, m) for jb in range(JROWS // 512) for m in range(DC)]
            wave1, rest = mgroups[:8], mgroups[8:]

            def m_ap(g):
                jb, m = g
                rs = slice(jb * 512, (jb + 1) * 512)
                ms = slice(m * P, (m + 1) * P)
                return rs, ms

            gidx = 0
            tiles = {
                g: psbig.tile([P, 512], F32, tag="ps", name=f"mps{g[1]}")
                for g in wave1
            }
            # kc-outer through layer KC-2 (consume the stream in arrival
            # order); the last layer runs group-major so each group stops
            # early and its extract staggers into the following matmuls.
            for kc in range(KC - 1):
                for term in range(3):
                    for g in wave1:
                        rs, ms = m_ap(g)
                        ks = slice(2 * kc, 2 * kc + 2)
                        lhs = (gl_sb if term == 2 else g8_sb)[:, ks, ms]
                        rhs = (xl_sb if term == 1 else x8_sb)[:, ks, rs]
                        mm_term(tiles[g], lhs, rhs, kc == 0 and term == 0, False)
            ks = slice(2 * (KC - 1), 2 * KC)
            for g in wave1:
                rs, ms = m_ap(g)
                for term in range(3):
                    lhs = (gl_sb if term == 2 else g8_sb)[:, ks, ms]
                    rhs = (xl_sb if term == 1 else x8_sb)[:, ks, rs]
                    mm_term(tiles[g], lhs, rhs, False, term == 2)
                jb, m = g
                with tc.high_priority():
                    extract(tiles[g], M8_sb[:, m, rs], Ml_sb[:, m, rs])
                gidx += 1
            for g in rest:
                rs, ms = m_ap(g)
                wps = psbig.tile([P, 512], F32, tag="ps")
                n = 0
                for kc in range(KC):
                    for term in range(3):
                        ksg = slice(2 * kc, 2 * kc + 2)
                        lhs = (gl_sb if term == 2 else g8_sb)[:, ksg, ms]
                        rhs = (xl_sb if term == 1 else x8_sb)[:, ksg, rs]
                        mm_term(wps, lhs, rhs, n == 0, n == 3 * KC - 1)
                        n += 1
                jb, m = g
                extract(wps, M8_sb[:, m, rs], Ml_sb[:, m, rs])
                if gidx - 8 < JC:
                    bias_group(gidx - 8)
                gidx += 1

            # ---- phase V: v = x @ Wv (own j; bv folded out on host) ----
            # Runs between B1 and C0 (its first consumer): by then all of its
            # DMA has long landed, so plain group order, no stalls.
            def phase_v():
                for j in range(JC):
                    js = slice(j * P, (j + 1) * P)
                    for ob in range(2):
                        os_ = slice(ob * 512, (ob + 1) * 512)
                        ps = psbig.tile([P, 512], F32, tag="ps")
                        n = 0
                        for term in range(3):
                            for kc in range(KC):
                                ks = slice(2 * kc, 2 * kc + 2)
                                lhs = (xl_sb if term == 2 else x8_sb)[:, ks, js]
                                rhs = (wvl_sb if term == 1 else wv8_sb)[:, ks, os_]
                                mm_term(ps, lhs, rhs, n == 0, n == 3 * KC - 1)
                                n += 1
                        extract(ps, v8_sb[:, j, os_], vl_sb[:, j, os_])

            # ---- phases B+C, software-pipelined over query blocks ----
            ahis, alos = {}, {}

            def phase_b(qb):
                qs = slice(qb * 512, (qb + 1) * 512)
                ahi = attnp.tile([P, JC, 512], E4, tag="ahi")
                alo = attnp.tile([P, JC, 512], E5, tag="alo")
                ahis[qb], alos[qb] = ahi, alo
                for j in range(JC):
                    js = slice(j * P, (j + 1) * P)
                    ps = psbig.tile([P, 512], F32, tag="ps")
                    mms = [
                        (term, kc)
                        for term in range(3)
                        for kc in range(KC)
                        if not (term == 1 and kc >= _KCB1)
                        and not (term == 2 and kc >= _KCB2)
                    ]
                    for n, (term, kc) in enumerate(mms):
                        ks = slice(2 * kc, 2 * kc + 2)
                        lhs = (Ml_sb if term == 2 else M8_sb)[:, ks, js]
                        rhs = (xl_sb if term == 1 else x8_sb)[:, ks, qs]
                        mm_term(ps, lhs, rhs, n == 0, n == len(mms) - 1)
                    a16 = a16p.tile([P, 512], F16, tag="a16")
                    nc.scalar.activation(
                        out=a16, in_=ps, func=AFT.Exp,
                        scale=float(SCALE), bias=bqk_sb[:, j : j + 1],
                    )
                    nc.scalar.activation(
                        out=ahi[:, j, :], in_=a16, func=AFT.Copy, scale=1.0
                    )
                    nc.vector.tensor_tensor(
                        out=alo[:, j, :], in0=a16, in1=ahi[:, j, :],
                        op=mybir.AluOpType.subtract,
                    )

            def phase_c(qb):
                ahi, alo = ahis.pop(qb), alos.pop(qb)
                for qc in range(4):
                    qls = slice(qc * P, (qc + 1) * P)
                    pdt = psbig.tile([P, 512], F32, tag="ps")
                    pd = pdt[:, 0:1]
                    for kc in range(KC):
                        nc.tensor.matmul(
                            pd, ahi[:, 2 * kc : 2 * kc + 2, qls],
                            ones_sb[:, 2 * kc : 2 * kc + 2, :],
                            start=(kc == 0), stop=False, perf_mode=DR,
                        )
                    for kc in range(KC):
                        nc.tensor.matmul(
                            pd, alo[:, 2 * kc : 2 * kc + 2, qls],
                            ones_sb[:, 2 * kc : 2 * kc + 2, :],
                            start=False, stop=(kc == KC - 1), perf_mode=DR,
                        )
                    qrow = qb * 512 + qc * P
                    di = qb * 4 + qc
                    nc.vector.tensor_copy(den_sb[:, di : di + 1], pd)
                    last = qb == 3 and qc == 3
                    for ob in range(2):
                        if last and ob == 1:
                            continue
                        os_ = slice(ob * 512, (ob + 1) * 512)
                        po = psbig.tile([P, 512], F32, tag="ps")
                        cmms = [
                            (term, kc)
                            for term in range(3)
                            for kc in range(KC)
                            if not (term == 1 and kc >= _KCC1)
                            and not (
                                term == 2 and kc >= _KCC2
                                and (ob == 1 or not _KCC2H)
                            )
                        ]
                        for n, (term, kc) in enumerate(cmms):
                            ks = slice(2 * kc, 2 * kc + 2)
                            lhs = (alo if term == 2 else ahi)[:, ks, qls]
                            rhs = (vl_sb if term == 1 else v8_sb)[:, ks, os_]
                            mm_term(po, lhs, rhs, n == 0, n == len(cmms) - 1)
                        o = outp.tile([P, 512], F16, tag="o")
                        # alternate copies between DVE and ACT: halves the
                        # serial copy chain at the kernel tail and balances
                        # elementwise load across engines during C phases
                        # producer engine also issues the DMA: keeps output
                        # DMA issue off the SP queue (which would serialize
                        # the kernel tail behind 650ns/DMA SEQ holds)
                        if ob == 0:
                            nc.vector.tensor_copy(o, po)
                            nc.sync.dma_start(out=pre_d[qrow : qrow + P, os_], in_=o)
                        else:
                            nc.scalar.activation(out=o, in_=po, func=AFT.Copy, scale=1.0)
                            nc.scalar.dma_start(out=pre_d[qrow : qrow + P, os_], in_=o)
                    if last:
                        # final 512 cols go as two 256-wide groups so the
                        # closing copy+DMA chain after the very last matmul
                        # is half-width (and the last DMA rides SP's shorter
                        # DGE delay)
                        for half in range(2):
                            os2 = slice(512 + half * 256, 512 + (half + 1) * 256)
                            po2 = psbig.tile([P, 256], F32, tag="ps")
                            cmms = [
                                (term, kc)
                                for term in range(3)
                                for kc in range(KC)
                                if not (term == 1 and kc >= _KCC1)
                                and not (term == 2 and kc >= _KCC2)
                            ]
                            for n, (term, kc) in enumerate(cmms):
                                ks = slice(2 * kc, 2 * kc + 2)
                                lhs = (alo if term == 2 else ahi)[:, ks, qls]
                                rhs = (vl_sb if term == 1 else v8_sb)[:, ks, os2]
                                mm_term(po2, lhs, rhs, n == 0, n == len(cmms) - 1)
                            o2 = outp.tile([P, 256], F16, tag="o2", bufs=2)
                            if half == 0:
                                nc.scalar.activation(
                                    out=o2, in_=po2, func=AFT.Copy, scale=1.0
                                )
                                nc.scalar.dma_start(
                                    out=pre_d[qrow : qrow + P, os2], in_=o2
                                )
                            else:
                                nc.vector.tensor_copy(o2, po2)
                                nc.sync.dma_start(
                                    out=pre_d[qrow : qrow + P, os2], in_=o2
                                )

            phase_b(0)
            phase_b(1)
            phase_v()
            phase_c(0)
            phase_b(2)
            phase_c(1)
            phase_b(3)
            phase_c(2)
            phase_c(3)
            nc.sync.dma_start(out=den_d, in_=den_sb)

    _split_excess_waits(nc)
    return nc


def _get_nc():
    if "nc" not in _CACHED:
        _CACHED["nc"] = _build()
    return _CACHED["nc"]


def _split8(a):
    hi = np.ascontiguousarray(a).astype(E4NP)
    lo = (a - hi.astype(np.float32)).astype(E5NP)
    return hi, np.ascontiguousarray(lo)


def _img(a):
    """[D, X] -> SBUF image [P, DC, X] (partition p holds row d = c*128+p)."""
    return np.ascontiguousarray(a.reshape(DC, P, -1).swapaxes(0, 1))


def kernel(x, Wq, bq, Wk, bk, Wv, bv):
    x = np.asarray(x, dtype=np.float32)
    Wq32 = np.asarray(Wq, np.float32)
    Wk32 = np.asarray(Wk, np.float32)
    bq32 = np.asarray(bq, np.float32)
    bk32 = np.asarray(bk, np.float32)
    # weight fusion: G^T = Wk Wq^T so scores = x G x^T; w = Wk bq; c0 = bq.bk
    g8, gl = _split8(Wk32 @ Wq32.T)
    g8, gl = _img(g8), _img(gl)
    wv8, wvl = _split8(np.asarray(Wv, np.float32))
    wv8, wvl = _img(wv8), _img(wvl)
    w8 = np.ascontiguousarray(
        (Wk32 @ bq32).reshape(DC, P).T.reshape(P, DC, 1)
    ).astype(E4NP)
    c0s = np.full(
        (P, 1), float(SCALE) * float(bq32 @ bk32) - ESHIFT, np.float32
    )
    bv32 = np.asarray(bv, np.float32).reshape(1, 1, D)

    in_maps = []
    for core in range(NCORES):
        b, h = core // 2, core % 2
        # own j rows first (j order is internal; q order is undone on gather)
        xb = np.roll(x[b], -h * JROWS, axis=0) if h else x[b]
        x8, xlo = _split8(np.ascontiguousarray(xb.T))  # [D, S]
        x8i, xli = _img(x8), _img(xlo)  # [P, DC, S]
        in_maps.append(
            {"x8o": np.ascontiguousarray(x8i[:, :, 0:JROWS]),
             "xlo": np.ascontiguousarray(xli[:, :, 0:JROWS]),
             "x8r": np.ascontiguousarray(x8i[:, :, JROWS:S]),
             "xlr": np.ascontiguousarray(xli[:, :, JROWS:S]),
             "g8": g8, "gl": gl, "wv8": wv8, "wvl": wvl,
             "w8": w8, "c0s": c0s}
        )

    res = run_bass_kernel_spmd(_get_nc(), in_maps, list(range(NCORES)))
    out = np.empty((B, S, D), np.float32)
    for b in range(B):
        r0, r1 = res.results[2 * b], res.results[2 * b + 1]
        pre = r0["pre"].astype(np.float32) + np.roll(
            r1["pre"].astype(np.float32), JROWS, axis=0
        )
        # den comes back packed [P, 16]: col qb*4+qc = rows qb*512+qc*128+p
        d0 = np.ascontiguousarray(r0["den"].T).reshape(S, 1)
        d1 = np.ascontiguousarray(r1["den"].T).reshape(S, 1)
        den = d0 + np.roll(d1, JROWS, axis=0)
        out[b] = pre / den
    out += bv32
    return out



# revision 46
# speedup vs baseline: 1.0528x; 1.0024x over previous
"""Single-head self-attention (B=4, S=2048, D=1024, fp32) on 8 trn2 NeuronCores.

Sharding: each core owns (batch b = core//2, sequence half h = core%2), as in
the fp16 baseline: Q/K are folded into one projection via G = Wq Wk^T
(scores = x G x^T), V is computed for own j rows only, and each core emits
partial softmax numerator `pre` and denominator `den` for its j half; the
host combines halves exactly: out = (pre0 + pre1)/(den0 + den1) + bv
(bv is pulled out of the device: sum_j a_j (v_j + bv) = sum_j a_j v_j +
den * bv, which reduces to "+ bv" after the division).

Speed comes from fp8 DoubleRow matmuls (2 fp8 rows per PE pass, K=256 per
instruction). e4m3 alone is too coarse (~2.5% rms), so every matmul operand
X is carried as a split pair X = X8 (e4m3) + Xl (e5m2 residual, natural
scale) and each product uses three cross terms accumulated in one PSUM
group:  A@B ~= A8@B8 + A8@Bl + Al@B8  (the dropped Al@Bl term is O(delta^2)).
The scores phase further drops the last of 4 K-chunks of its key-side
residual term (KCB2=3) — measured end-to-end error 1.25e-2 (max metric,
rms 9.0e-3) vs the 2e-2 gate, saving 32 big matmuls/core. x/G/Wv
splits are host-prepared; M and attn splits are extracted on device from
PSUM via ACT copy (hi, e4m3) and DVE subtract (lo, e5m2). exp is shifted by
-3*ln2 (folded into the constant bias) so attn stays below e4m3 max; the
pre/den ratio is shift-invariant.

Schedule: a single 8-bank PSUM ring serves every phase (den/bias slice a
[P,1] column out of a full-bank tile). M runs first: wave 1 = all 8 jb0
groups kc-outer so matmuls chase the interleaved (g8, x8, xl, gl) quarter
stream in arrival order, with the last K-layer group-major so extracts
stagger; the remaining 8 jb1 groups run group-major with bias groups
interleaved after their extracts. B (scores) and C (attn @ V) are
software-pipelined (B0 B1 V C0 B2 C1 B3 C2 C3) to hide the exp/split
extraction latency; B0/B1 read only own-half q columns so the other x half
may arrive as late as B2. den values accumulate in a [P,16] SBUF tile and
leave in one end-of-kernel DMA; C3's last 512 output columns go as two
256-wide groups so the closing copy+DMA chain after the final matmul is
half-width. A few throwaway warm matmuls (NWARM=4) keep the first real
matmuls' cost-model visits past the 3us p-state ramp.
"""

import numpy as np
import ml_dtypes

import concourse.bass as bass
import concourse.mybir as mybir
import concourse.tile as tile
import concourse.tile_scheduler as _tsch
import concourse.tile_sem_assignment as _tsem
from concourse.bass_utils import run_bass_kernel_spmd

# 4 HWDGE queue semaphores instead of 8: fewer live sems shortens the
# end-of-kernel drain wait chain; 4 queues still keep DMA issue pipelined.
_tsch.NUM_HWDGE_SEMS = 4
_tsem.NUM_HWDGE_SEMS = 4

E4 = mybir.dt.float8e4
E5 = mybir.dt.float8e5
F16 = mybir.dt.float16
F32 = mybir.dt.float32
AFT = mybir.ActivationFunctionType
DR = mybir.MatmulPerfMode.DoubleRow
E4NP = ml_dtypes.float8_e4m3
E5NP = ml_dtypes.float8_e5m2

B, S, D = 4, 2048, 1024
NCORES = 8
P = 128
DC = D // P            # 8 contraction chunks of 128
KC = DC // 2           # 4 DoubleRow K-chunks of 256
JROWS = S // 2         # 1024 own k/v rows per core
JC = JROWS // P        # 8 own j chunks
QB = S // 512          # 4 query col-blocks of 512
SCALE = 1.0 / np.sqrt(np.float32(D))  # 1/32
ESHIFT = 3.0 * np.log(2.0)  # keep exp(score) < e4m3 max (448)

_CACHED = {}


def _split_excess_waits(nc, max_waits=1):
    """walrus in this env rejects >1 sync-wait per instruction (Drain at Tile
    exit carries one per live semaphore); move extras onto same-engine NOPs."""
    for f in nc.m.functions:
        for bb in f.blocks:
            new_list, changed = [], False
            for ins in bb.instructions:
                si = getattr(ins, "sync_info", None)
                ow = list(si.on_wait) if si and si.on_wait else []
                if len(ow) > max_waits:
                    extra, keep = ow[:-max_waits], ow[-max_waits:]
                    for k, w in enumerate(extra):
                        new_list.append(
                            mybir.InstNoOp(
                                name=f"{ins.name}_ws{k}",
                                engine=ins.engine,
                                sync_info=mybir.SyncInfo(on_wait=[w], on_update=[]),
                                bass_nofuse=True,
                            )
                        )
                    si.on_wait = keep
                    changed = True
                new_list.append(ins)
            if changed:
                bb.instructions = new_list


def _build():
    nc = bass.Bass("TRN2", target_bir_lowering=False, debug=False, num_devices=NCORES)

    # All big inputs are host pre-imaged to the SBUF layout [P, DC, X]
    # (partition p holds d = c*128+p) so each needs only two large DMAs:
    # HWDGE descriptor-generation time (625ns per DMA instruction, serialized)
    # gates the input stream, not bytes.
    x8o_d = nc.dram_tensor("x8o", [P, DC, JROWS], E4, kind="ExternalInput").ap()
    xlo_d = nc.dram_tensor("xlo", [P, DC, JROWS], E5, kind="ExternalInput").ap()
    x8r_d = nc.dram_tensor("x8r", [P, DC, JROWS], E4, kind="ExternalInput").ap()
    xlr_d = nc.dram_tensor("xlr", [P, DC, JROWS], E5, kind="ExternalInput").ap()
    g8_d = nc.dram_tensor("g8", [P, DC, D], E4, kind="ExternalInput").ap()
    gl_d = nc.dram_tensor("gl", [P, DC, D], E5, kind="ExternalInput").ap()
    wv8_d = nc.dram_tensor("wv8", [P, DC, D], E4, kind="ExternalInput").ap()
    wvl_d = nc.dram_tensor("wvl", [P, DC, D], E5, kind="ExternalInput").ap()
    w8_d = nc.dram_tensor("w8", [P, DC, 1], E4, kind="ExternalInput").ap()
    c0s_d = nc.dram_tensor("c0s", [P, 1], F32, kind="ExternalInput").ap()
    pre_d = nc.dram_tensor("pre", [S, D], F16, kind="ExternalOutput").ap()
    # den packed [P, 16]: col qb*4+qc holds den for query rows
    # qb*512+qc*128+p — one end-of-kernel DMA instead of 16 tiny ones
    den_d = nc.dram_tensor("den", [P, QB * 4], F32, kind="ExternalOutput").ap()

    with tile.TileContext(nc) as tc:
        with (
            tc.tile_pool(name="persist", bufs=1) as persist,
            tc.tile_pool(name="outp", bufs=16) as outp,
            tc.tile_pool(name="attnp", bufs=2) as attnp,
            tc.tile_pool(name="a16p", bufs=4) as a16p,
            tc.tile_pool(name="ps", bufs=8, space="PSUM") as psbig,
        ):
            # ---- persistent SBUF ----
            x8_sb = persist.tile([P, DC, S], E4, tag="x8")
            xl_sb = persist.tile([P, DC, S], E5, tag="xl")
            M8_sb = persist.tile([P, DC, JROWS], E4, tag="M8")
            Ml_sb = persist.tile([P, DC, JROWS], E5, tag="Ml")
            v8_sb = persist.tile([P, JC, D], E4, tag="v8")
            vl_sb = persist.tile([P, JC, D], E5, tag="vl")
            g8_sb = persist.tile([P, DC, D], E4, tag="g8")
            gl_sb = persist.tile([P, DC, D], E5, tag="gl")
            wv8_sb = persist.tile([P, DC, D], E4, tag="wv8")
            wvl_sb = persist.tile([P, DC, D], E5, tag="wvl")
            bqk_sb = persist.tile([P, JC], F32, tag="bqk")
            w8_sb = persist.tile([P, DC, 1], E4, tag="w8")
            c0s_sb = persist.tile([P, 1], F32, tag="c0s")
            ones_sb = persist.tile([P, JC, 1], E4, tag="ones")
            den_sb = persist.tile([P, QB * 4], F32, tag="den")

            nc.vector.memset(ones_sb, 1.0)

            # A few throwaway warm matmuls execute inside the initial DMA
            # wait; they push the first real matmuls' cost-model visit times
            # past the 3us p-state ramp so M runs at full PE clock.
            import os as _os
            # how many of B's 4 K-chunks keep the q-side (term 1) and
            # key-side (term 2) residual corrections. 3/4 drops a quarter of
            # the q-side correction: measured end-to-end error 1.50e-02 vs
            # the 2e-2 gate (3.52e-03 at 4/4), for 32 fewer big matmuls.
            _KCB1 = int(_os.environ.get("KCB1", "4"))
            _KCB2 = int(_os.environ.get("KCB2", "3"))
            _KCC1 = int(_os.environ.get("KCC1", "4"))
            _KCC2 = int(_os.environ.get("KCC2", "4"))
            _KCC2H = int(_os.environ.get("KCC2H", "0"))
            nwarm = int(_os.environ.get("NWARM", "4"))
            if nwarm:
                warm_sb = persist.tile([P, 512], F16, tag="warm")
                nc.vector.memset(warm_sb, 0.0)
                pw = psbig.tile([P, 512], F32, tag="ps")
                for _ in range(nwarm):
                    nc.tensor.matmul(
                        pw, warm_sb[:, 0:P], warm_sb, start=True, stop=True
                    )

            # DMA order == consumption order: M's four operand tensors
            # (g8, x8 own, xl own, gl) stream interleaved at quarter
            # granularity so M's kc-layers start as chunks land; the Wv pair
            # arrives during B0/B1 (V runs after B1); the other x half is
            # first read by B2, much later.
            for h in range(4):
                hs = slice(h * 2, h * 2 + 2)
                nc.sync.dma_start(out=g8_sb[:, hs, :], in_=g8_d[:, hs, :])
                nc.sync.dma_start(out=x8_sb[:, hs, 0:JROWS], in_=x8o_d[:, hs, :])
                nc.sync.dma_start(out=xl_sb[:, hs, 0:JROWS], in_=xlo_d[:, hs, :])
                nc.sync.dma_start(out=gl_sb[:, hs, :], in_=gl_d[:, hs, :])
            nc.sync.dma_start(out=w8_sb, in_=w8_d[:, :, :])
            nc.sync.dma_start(out=c0s_sb, in_=c0s_d[:, :])
            H = DC // 2
            for h in range(2):
                hs = slice(h * H, (h + 1) * H)
                nc.sync.dma_start(out=wv8_sb[:, hs, :], in_=wv8_d[:, hs, :])
                nc.sync.dma_start(out=wvl_sb[:, hs, :], in_=wvl_d[:, hs, :])
            for h in range(2):
                hs = slice(h * H, (h + 1) * H)
                nc.sync.dma_start(out=x8_sb[:, hs, JROWS:S], in_=x8r_d[:, hs, :])
                nc.sync.dma_start(out=xl_sb[:, hs, JROWS:S], in_=xlr_d[:, hs, :])

            def mm_term(ps, lhs, rhs, first, last):
                nc.tensor.matmul(ps, lhs, rhs, start=first, stop=last, perf_mode=DR)

            def extract(ps, hi_ap, lo_ap, halves=1):
                # halves=2 splits the copy/sub into column halves so the
                # PSUM slot releases after the shorter chained half
                w = ps.shape[-1] // halves
                for hh in range(halves):
                    sl = slice(hh * w, (hh + 1) * w)
                    nc.scalar.activation(
                        out=hi_ap[..., sl], in_=ps[:, sl], func=AFT.Copy, scale=1.0
                    )
                    nc.vector.tensor_tensor(
                        out=lo_ap[..., sl], in0=ps[:, sl], in1=hi_ap[..., sl],
                        op=mybir.AluOpType.subtract,
                    )

            # bias[j] = (x_j.w)*SCALE + (bq.bk*SCALE - 3ln2), [j, 1]; its
            # tiny psum groups are interleaved between M group extractions so
            # their pool-recycle latency hides under matmul work.
            def bias_group(j):
                js = slice(j * P, (j + 1) * P)
                pbt = psbig.tile([P, 512], F32, tag="ps")
                pb = pbt[:, 0:1]
                for kc in range(KC):
                    nc.tensor.matmul(
                        pb,
                        x8_sb[:, 2 * kc : 2 * kc + 2, js],
                        w8_sb[:, 2 * kc : 2 * kc + 2, :],
                        start=(kc == 0),
                        stop=(kc == KC - 1),
                        perf_mode=DR,
                    )
                nc.vector.tensor_scalar(
                    out=bqk_sb[:, j : j + 1], in0=pb,
                    scalar1=float(SCALE), scalar2=c0s_sb[:, 0:1],
                    op0=mybir.AluOpType.mult, op1=mybir.AluOpType.add,
                )

            # ---- phase M: M[d, j] = sum_d' G[d,d'] x[j,d'] (own j) ----
            # Wave 1 = all 8 jb0 groups, kc-outer through layer KC-2 so each
            # K-chunk's matmuls run as the chunk quad (g8, x8, xl, gl) lands;
            # the last layer runs group-major so extracts stagger into the
            # following groups' matmuls instead of bunching at a wave barrier.
            # Remaining 8 groups (jb1) run group-major: extract right after
            # each group's 12 matmuls, pipelined with the next group.
            mgroups = [(jb, m) for jb in range(JROWS // 512) for m in range(DC)]
            wave1, rest = mgroups[:8], mgroups[8:]

            def m_ap(g):
                jb, m = g
                rs = slice(jb * 512, (jb + 1) * 512)
                ms = slice(m * P, (m + 1) * P)
                return rs, ms

            gidx = 0
            tiles = {
                g: psbig.tile([P, 512], F32, tag="ps", name=f"mps{g[1]}")
                for g in wave1
            }
            # kc-outer through layer KC-2 (consume the stream in arrival
            # order); the last layer runs group-major so each group stops
            # early and its extract staggers into the following matmuls.
            for kc in range(KC - 1):
                for term in range(3):
                    for g in wave1:
                        rs, ms = m_ap(g)
                        ks = slice(2 * kc, 2 * kc + 2)
                        lhs = (gl_sb if term == 2 else g8_sb)[:, ks, ms]
                        rhs = (xl_sb if term == 1 else x8_sb)[:, ks, rs]
                        mm_term(tiles[g], lhs, rhs, kc == 0 and term == 0, False)
            ks = slice(2 * (KC - 1), 2 * KC)
            for g in wave1:
                rs, ms = m_ap(g)
                for term in range(3):
                    lhs = (gl_sb if term == 2 else g8_sb)[:, ks, ms]
                    rhs = (xl_sb if term == 1 else x8_sb)[:, ks, rs]
                    mm_term(tiles[g], lhs, rhs, False, term == 2)
                jb, m = g
                with tc.high_priority():
                    extract(tiles[g], M8_sb[:, m, rs], Ml_sb[:, m, rs])
                gidx += 1
            for g in rest:
                rs, ms = m_ap(g)
                wps = psbig.tile([P, 512], F32, tag="ps")
                n = 0
                for kc in range(KC):
                    for term in range(3):
                        ksg = slice(2 * kc, 2 * kc + 2)
                        lhs = (gl_sb if term == 2 else g8_sb)[:, ksg, ms]
                        rhs = (xl_sb if term == 1 else x8_sb)[:, ksg, rs]
                        mm_term(wps, lhs, rhs, n == 0, n == 3 * KC - 1)
                        n += 1
                jb, m = g
                extract(wps, M8_sb[:, m, rs], Ml_sb[:, m, rs])
                if gidx - 8 < JC:
                    bias_group(gidx - 8)
                gidx += 1

            # ---- phase V: v = x @ Wv (own j; bv folded out on host) ----
            # Runs between B1 and C0 (its first consumer): by then all of its
            # DMA has long landed, so plain group order, no stalls.
            def phase_v():
                for j in range(JC):
                    js = slice(j * P, (j + 1) * P)
                    for ob in range(2):
                        os_ = slice(ob * 512, (ob + 1) * 512)
                        ps = psbig.tile([P, 512], F32, tag="ps")
                        n = 0
                        for term in range(3):
                            for kc in range(KC):
                                ks = slice(2 * kc, 2 * kc + 2)
                                lhs = (xl_sb if term == 2 else x8_sb)[:, ks, js]
                                rhs = (wvl_sb if term == 1 else wv8_sb)[:, ks, os_]
                                mm_term(ps, lhs, rhs, n == 0, n == 3 * KC - 1)
                                n += 1
                        extract(ps, v8_sb[:, j, os_], vl_sb[:, j, os_])

            # ---- phases B+C, software-pipelined over query blocks ----
            ahis, alos = {}, {}

            def phase_b(qb):
                qs = slice(qb * 512, (qb + 1) * 512)
                ahi = attnp.tile([P, JC, 512], E4, tag="ahi")
                alo = attnp.tile([P, JC, 512], E5, tag="alo")
                ahis[qb], alos[qb] = ahi, alo
                for j in range(JC):
                    js = slice(j * P, (j + 1) * P)
                    ps = psbig.tile([P, 512], F32, tag="ps")
                    mms = [
                        (term, kc)
                        for term in range(3)
                        for kc in range(KC)
                        if not (term == 1 and kc >= _KCB1)
                        and not (term == 2 and kc >= _KCB2)
                    ]
                    for n, (term, kc) in enumerate(mms):
                        ks = slice(2 * kc, 2 * kc + 2)
                        lhs = (Ml_sb if term == 2 else M8_sb)[:, ks, js]
                        rhs = (xl_sb if term == 1 else x8_sb)[:, ks, qs]
                        mm_term(ps, lhs, rhs, n == 0, n == len(mms) - 1)
                    a16 = a16p.tile([P, 512], F16, tag="a16")
                    nc.scalar.activation(
                        out=a16, in_=ps, func=AFT.Exp,
                        scale=float(SCALE), bias=bqk_sb[:, j : j + 1],
                    )
                    nc.scalar.activation(
                        out=ahi[:, j, :], in_=a16, func=AFT.Copy, scale=1.0
                    )
                    nc.vector.tensor_tensor(
                        out=alo[:, j, :], in0=a16, in1=ahi[:, j, :],
                        op=mybir.AluOpType.subtract,
                    )

            def phase_c(qb):
                ahi, alo = ahis.pop(qb), alos.pop(qb)
                for qc in range(4):
                    qls = slice(qc * P, (qc + 1) * P)
                    pdt = psbig.tile([P, 512], F32, tag="ps")
                    pd = pdt[:, 0:1]
                    for kc in range(KC):
                        nc.tensor.matmul(
                            pd, ahi[:, 2 * kc : 2 * kc + 2, qls],
                            ones_sb[:, 2 * kc : 2 * kc + 2, :],
                            start=(kc == 0), stop=False, perf_mode=DR,
                        )
                    for kc in range(KC):
                        nc.tensor.matmul(
                            pd, alo[:, 2 * kc : 2 * kc + 2, qls],
                            ones_sb[:, 2 * kc : 2 * kc + 2, :],
                            start=False, stop=(kc == KC - 1), perf_mode=DR,
                        )
                    qrow = qb * 512 + qc * P
                    di = qb * 4 + qc
                    nc.vector.tensor_copy(den_sb[:, di : di + 1], pd)
                    last = qb == 3 and qc == 3
                    for ob in range(2):
                        if last and ob == 1:
                            continue
                        os_ = slice(ob * 512, (ob + 1) * 512)
                        po = psbig.tile([P, 512], F32, tag="ps")
                        cmms = [
                            (term, kc)
                            for term in range(3)
                            for kc in range(KC)
                            if not (term == 1 and kc >= _KCC1)
                            and not (
                                term == 2 and kc >= _KCC2
                                and (ob == 1 or not _KCC2H)
                            )
                        ]
                        for n, (term, kc) in enumerate(cmms):
                            ks = slice(2 * kc, 2 * kc + 2)
                            lhs = (alo if term == 2 else ahi)[:, ks, qls]
                            rhs = (vl_sb if term == 1 else v8_sb)[:, ks, os_]
                            mm_term(po, lhs, rhs, n == 0, n == len(cmms) - 1)
                        o = outp.tile([P, 512], F16, tag="o")
                        # alternate copies between DVE and ACT: halves the
                        # serial copy chain at the kernel tail and balances
                        # elementwise load across engines during C phases
                        # producer engine also issues the DMA: keeps output
                        # DMA issue off the SP queue (which would serialize
                        # the kernel tail behind 650ns/DMA SEQ holds)
                        if ob == 0:
                            nc.vector.tensor_copy(o, po)
                            nc.sync.dma_start(out=pre_d[qrow : qrow + P, os_], in_=o)
                        else:
                            nc.scalar.activation(out=o, in_=po, func=AFT.Copy, scale=1.0)
                            nc.scalar.dma_start(out=pre_d[qrow : qrow + P, os_], in_=o)
                    if last:
                        # final 512 cols go as two 256-wide groups so the
                        # closing copy+DMA chain after the very last matmul
                        # is half-width (and the last DMA rides SP's shorter
                        # DGE delay)
                        for half in range(2):
                            os2 = slice(512 + half * 256, 512 + (half + 1) * 256)
                            po2 = psbig.tile([P, 256], F32, tag="ps")
                            cmms = [
                                (term, kc)
                                for term in range(3)
                                for kc in range(KC)
                                if not (term == 1 and kc >= _KCC1)
                                and not (term == 2 and kc >= _KCC2)
                            ]
                            for n, (term, kc) in enumerate(cmms):
                                ks = slice(2 * kc, 2 * kc + 2)
                                lhs = (alo if term == 2 else ahi)[:, ks, qls]
                                rhs = (vl_sb if term == 1 else v8_sb)[:, ks, os2]
                                mm_term(po2, lhs, rhs, n == 0, n == len(cmms) - 1)
                            o2 = outp.tile([P, 256], F16, tag="o2", bufs=2)
                            if half == 0:
                                nc.scalar.activation(
                                    out=o2, in_=po2, func=AFT.Copy, scale=1.0
                                )
                                # idle Pool/SWDGE path keeps HWDGE free for
                                # the final half's SP DMA
                                nc.gpsimd.dma_start(
                                    out=pre_d[qrow : qrow + P, os2], in_=o2
                                )
                            else:
                                nc.vector.tensor_copy(o2, po2)
                                nc.sync.dma_start(
                                    out=pre_d[qrow : qrow + P, os2], in_=o2
                                )

            phase_b(0)
            phase_b(1)
            phase_v()
            phase_c(0)
            phase_b(2)
            phase_c(1)
            phase_b(3)
            phase_c(2)
            phase_c(3)
            nc.sync.dma_start(out=den_d, in_=den_sb)

    _split_excess_waits(nc)
    return nc


def _get_nc():
    if "nc" not in _CACHED:
        _CACHED["nc"] = _build()
    return _CACHED["nc"]


def _split8(a):
    hi = np.ascontiguousarray(a).astype(E4NP)
    lo = (a - hi.astype(np.float32)).astype(E5NP)
    return hi, np.ascontiguousarray(lo)


def _img(a):
    """[D, X] -> SBUF image [P, DC, X] (partition p holds row d = c*128+p)."""
    return np.ascontiguousarray(a.reshape(DC, P, -1).swapaxes(0, 1))


def kernel(x, Wq, bq, Wk, bk, Wv, bv):
    x = np.asarray(x, dtype=np.float32)
    Wq32 = np.asarray(Wq, np.float32)
    Wk32 = np.asarray(Wk, np.float32)
    bq32 = np.asarray(bq, np.float32)
    bk32 = np.asarray(bk, np.float32)
    # weight fusion: G^T = Wk Wq^T so scores = x G x^T; w = Wk bq; c0 = bq.bk
    g8, gl = _split8(Wk32 @ Wq32.T)
    g8, gl = _img(g8), _img(gl)
    wv8, wvl = _split8(np.asarray(Wv, np.float32))
    wv8, wvl = _img(wv8), _img(wvl)
    w8 = np.ascontiguousarray(
        (Wk32 @ bq32).reshape(DC, P).T.reshape(P, DC, 1)
    ).astype(E4NP)
    c0s = np.full(
        (P, 1), float(SCALE) * float(bq32 @ bk32) - ESHIFT, np.float32
    )
    bv32 = np.asarray(bv, np.float32).reshape(1, 1, D)

    in_maps = []
    for core in range(NCORES):
        b, h = core // 2, core % 2
        # own j rows first (j order is internal; q order is undone on gather)
        xb = np.roll(x[b], -h * JROWS, axis=0) if h else x[b]
        x8, xlo = _split8(np.ascontiguousarray(xb.T))  # [D, S]
        x8i, xli = _img(x8), _img(xlo)  # [P, DC, S]
        in_maps.append(
            {"x8o": np.ascontiguousarray(x8i[:, :, 0:JROWS]),
             "xlo": np.ascontiguousarray(xli[:, :, 0:JROWS]),
             "x8r": np.ascontiguousarray(x8i[:, :, JROWS:S]),
             "xlr": np.ascontiguousarray(xli[:, :, JROWS:S]),
             "g8": g8, "gl": gl, "wv8": wv8, "wvl": wvl,
             "w8": w8, "c0s": c0s}
        )

    res = run_bass_kernel_spmd(_get_nc(), in_maps, list(range(NCORES)))
    out = np.empty((B, S, D), np.float32)
    for b in range(B):
        r0, r1 = res.results[2 * b], res.results[2 * b + 1]
        pre = r0["pre"].astype(np.float32) + np.roll(
            r1["pre"].astype(np.float32), JROWS, axis=0
        )
        # den comes back packed [P, 16]: col qb*4+qc = rows qb*512+qc*128+p
        d0 = np.ascontiguousarray(r0["den"].T).reshape(S, 1)
        d1 = np.ascontiguousarray(r1["den"].T).reshape(S, 1)
        den = d0 + np.roll(d1, JROWS, axis=0)
        out[b] = pre / den
    out += bv32
    return out

